# revision 13
# baseline (speedup 1.0000x reference)
"""GraphConv (DGL norm='both', 3 layers) on 8 trn2 NeuronCores.

Sharding: destination nodes (and their edges) are sharded across the 8
cores. Each layer, a core gathers the source rows of its edge shard from
a replicated bf16 node table with dma_gather (SPAN_CHUNKS*128-row spans,
4 SWDGE queues), reduces each 128-edge chunk into a per-dst-block [f, d]
PSUM accumulator via a weighted one-hot matmul (weights carry
ns[src]*nd[dst], so the full symmetric normalization folds into the
segment sum), applies W + bias + relu on the dst shard, and AllGathers
the new bf16 table for the next layer.

The weighted one-hot matrices are data (fixed per graph), so they are
precomputed on the host as one bf16 DRAM stream [128, nchunks*128] and
DMA'd in per span — no per-chunk vector work on device. All tables and
matmul operands are bf16 (fp32 PSUM accumulation); the final output is
fp32.

int16 gather indices only address 32768 rows, so the padded node table
(8 shards x 12544 rows = 100352) is split into 4 windows; each dst
block's edges are bucketed by (window) with a per-(block, window) chunk
count fixed across cores at compile time (schedule rebuilt per call
from the actual graph).
"""

import os
import numpy as np
import ml_dtypes

BF16 = ml_dtypes.bfloat16

N_NODES = 100000
F_IN = 128
F_HID = 128
F_OUT = 64
NCORES = 8
P = 128
SHARD = N_NODES // NCORES          # 12500 dst nodes per core
BLOCKS = (SHARD + P - 1) // P      # 98 dst blocks of 128
SHARD_PAD = BLOCKS * P             # 12544 rows per shard in the table
TAB_PAD = NCORES * SHARD_PAD       # 100352 rows in the gathered table
WIN = 32768                        # int16 index window
NWIN = (TAB_PAD + WIN - 1) // WIN  # 4
SPAN_CHUNKS = 8                    # chunks (of 128 edges) per dma_gather
SPAN_COLS = SPAN_CHUNKS * P        # one-hot columns per span tile
SPAN = SPAN_CHUNKS * P             # rows per dma_gather

LAST_EXEC_NS = None
LAST_RESULTS = None


def _preprocess(src, dst):
    """Build the static schedule and per-core slot arrays.

    Returns (cbr, gidx, onehot):
      cbr    [BLOCKS, NWIN] chunk count per (block, window), shared by cores
      gidx   [NCORES, 128, nslots//16] int16 wrapped gather indices
      onehot [NCORES, 128, nchunks*128] bf16 weighted one-hot blocks:
             onehot[c][p, cg*128 + j] = ns[src]*nd[dst] if slot p of chunk
             cg has local dst j, else 0.
    Slot streams are window-major: for w in windows, for b in blocks,
    cbr[b, w] chunks of 128 edges.
    """
    src = np.asarray(src).astype(np.int64).ravel()
    dst = np.asarray(dst).astype(np.int64).ravel()
    E = src.shape[0]
    deg_out = np.bincount(src, minlength=N_NODES)
    deg_in = np.bincount(dst, minlength=N_NODES)
    ns = np.power(np.maximum(deg_out, 1.0), -0.5).astype(np.float32)
    nd = np.power(np.maximum(deg_in, 1.0), -0.5).astype(np.float32)

    psrc = (src // SHARD) * SHARD_PAD + (src % SHARD)  # padded table row
    win = psrc // WIN
    core = dst // SHARD
    blk = (dst - core * SHARD) // P

    # sort edges by (core, window, block) to match the stream layout
    key = (core * NWIN + win) * BLOCKS + blk
    order = np.argsort(key, kind="stable")
    s_key = key[order]
    s_psrc = psrc[order]
    s_widx = (s_psrc - win[order] * WIN).astype(np.int16)
    s_loc = dst[order] - core[order] * SHARD
    s_dstl = (s_loc % P).astype(np.int64)
    s_w = (ns[src[order]] * nd[dst[order]]).astype(np.float32)

    counts = np.bincount(s_key, minlength=NCORES * NWIN * BLOCKS)
    counts = counts.reshape(NCORES, NWIN, BLOCKS)
    cbr = np.ceil(counts.max(axis=0) / P).astype(np.int64).T  # [BLOCKS, NWIN]
    cbr = np.maximum(cbr, 1)
    nchunks = int(cbr.sum())
    nslots = nchunks * P

    # slot offset of each (window, block) cell in the stream
    cell_chunk_off = np.zeros((NWIN, BLOCKS), np.int64)
    off = 0
    for w in range(NWIN):
        for b in range(BLOCKS):
            cell_chunk_off[w, b] = off
            off += cbr[b, w]

    starts = np.concatenate([[0], np.cumsum(counts.reshape(-1))[:-1]])
    pos = np.arange(E, dtype=np.int64) - starts[s_key]
    slot = cell_chunk_off[win[order], blk[order]] * P + pos
    # sanity: every edge fits its cell
    assert (pos < cbr[blk[order], win[order]] * P).all()

    gidx_flat = np.zeros((NCORES, nslots), dtype=np.int16)
    flat = core[order] * nslots + slot
    gidx_flat.reshape(-1)[flat] = s_widx

    # weighted one-hot stream, pre-tiled per gather span so each span's
    # block is contiguous in DRAM: [NCORES, nspan*128, SPAN_COLS].
    # Span gs covers chunks [woff[w]+s*SPAN_CHUNKS, ...); slot p of chunk
    # cg lands at row gs*128 + p, column (chunk-within-span)*128 + dstl.
    wlen = cbr.sum(axis=0)                              # [NWIN]
    woff = np.concatenate([[0], np.cumsum(wlen)])
    nspan_w = -(-wlen // SPAN_CHUNKS)
    spanoff = np.concatenate([[0], np.cumsum(nspan_w)])
    nspan = int(spanoff[-1])

    p_of = slot % P
    cg_of = slot // P
    w_of = np.searchsorted(woff[1:], cg_of, side="right")
    cw_of = cg_of - woff[w_of]
    s_of = cw_of // SPAN_CHUNKS
    gs_of = spanoff[w_of] + s_of
    col_of = (cw_of - s_of * SPAN_CHUNKS) * P + s_dstl
    onehot = np.zeros((NCORES, nspan * P, SPAN_COLS), dtype=BF16)
    onehot[core[order], gs_of * P + p_of, col_of] = s_w.astype(BF16)

    # gather-index wrap: logical idx i -> [i%16 + 16*rep, i//16]
    gw = gidx_flat.reshape(NCORES, nslots // 16, 16)
    gidx = np.ascontiguousarray(np.tile(gw.transpose(0, 2, 1), (1, 8, 1)))

    return cbr, gidx, onehot


def _build_program(cbr):
    import concourse.bacc as bacc
    import concourse.tile as tile
    from concourse import mybir

    f32 = mybir.dt.float32
    bf16 = mybir.dt.bfloat16
    i16 = mybir.dt.int16
    AF = mybir.ActivationFunctionType

    nchunks = int(cbr.sum())
    nslots = nchunks * P
    # per-window stream extents (in chunks)
    wlen = [int(cbr[:, w].sum()) for w in range(NWIN)]
    woff = np.concatenate([[0], np.cumsum(wlen)]).astype(int)
    nspan_w = [-(-wlen[w] // SPAN_CHUNKS) for w in range(NWIN)]
    spanoff = np.concatenate([[0], np.cumsum(nspan_w)]).astype(int)
    nspan = int(spanoff[-1])
    wrows = [min(WIN, TAB_PAD - w * WIN) for w in range(NWIN)]
    # chunk offset of cell (w, b) within the global stream
    cell_off = np.zeros((NWIN, BLOCKS), np.int64)
    off = 0
    for w in range(NWIN):
        for b in range(BLOCKS):
            cell_off[w, b] = off
            off += cbr[b, w]

    # 64 KiB/partition descriptor carveout -> 4096-descriptor rings per
    # SWDGE queue, so two 2048-row gathers can be in flight per queue
    # (the 16 KiB default ring fits only ONE 1024-row gather, serializing
    # descriptor generation with DMA drain).
    nc = bacc.Bacc("TRN2", target_bir_lowering=False, debug=False,
                   num_devices=NCORES, num_swdge_queues=4,
                   dynamic_dma_scratch_size=65536)

    feats = nc.dram_tensor("featpad", [TAB_PAD, F_IN], bf16,
                           kind="ExternalInput")
    w_d = [nc.dram_tensor(f"W{i}", [F_IN, fo], bf16, kind="ExternalInput")
           for i, fo in enumerate([F_HID, F_HID, F_OUT])]
    b_d = [nc.dram_tensor(f"b{i}", [fo], f32, kind="ExternalInput")
           for i, fo in enumerate([F_HID, F_HID, F_OUT])]
    gidx_d = nc.dram_tensor("gidx", [P, nslots // 16], i16,
                            kind="ExternalInput")
    oh_d = nc.dram_tensor("onehot", [nspan * P, SPAN_COLS], bf16,
                          kind="ExternalInput")
    ident_d = nc.dram_tensor("identity", [P, P], bf16, kind="ExternalInput")
    out_d = nc.dram_tensor("out", [SHARD, F_OUT], f32, kind="ExternalOutput")

    with tile.TileContext(nc) as tc:
        with (
            tc.tile_pool(name="const", bufs=1) as cpool,
            tc.tile_pool(name="gs", bufs=3) as gpool,
            tc.tile_pool(name="oh", bufs=3) as opool,
            tc.tile_pool(name="mid", bufs=3) as mpool,
            tc.tile_pool(name="ps", bufs=2, space="PSUM") as pspool,
            tc.tile_pool(name="dram", bufs=1, space="DRAM") as dpool,
        ):
            ident_sb = cpool.tile([P, P], bf16, tag="ident")
            nc.sync.dma_start(out=ident_sb[:], in_=ident_d[:])
            w_sb, b_sb = [], []
            for i, fo in enumerate([F_HID, F_HID, F_OUT]):
                t = cpool.tile([F_IN, fo], bf16, tag=f"w{i}")
                nc.sync.dma_start(out=t[:], in_=w_d[i][:])
                w_sb.append(t)
                t = cpool.tile([fo, 1], f32, tag=f"b{i}")
                nc.sync.dma_start(out=t[:], in_=b_d[i][:, None])
                b_sb.append(t)
            gidx_sb = cpool.tile([P, nslots // 16], i16, tag="gidx")
            nc.sync.dma_start(out=gidx_sb[:], in_=gidx_d[:])

            ag_in = dpool.tile([SHARD_PAD, F_HID], bf16, tag="ag_in")
            hf0 = dpool.tile([TAB_PAD, F_HID], bf16, tag="hf0",
                             addr_space="Shared")
            hf1 = dpool.tile([TAB_PAD, F_HID], bf16, tag="hf1",
                             addr_space="Shared")
            hf = [hf0, hf1]

            def layer(li, table_ap, fo, relu):
                # spans: per window, chop its chunk stream into
                # <=SPAN_CHUNKS-chunk gathers; span_tiles[w] maps span
                # index -> (gather tile, one-hot tile)
                qn = [0]
                span_tiles = [{} for _ in range(NWIN)]

                def ensure_span(w, s):
                    if s in span_tiles[w]:
                        return
                    c0 = s * SPAN_CHUNKS
                    ck = min(SPAN_CHUNKS, wlen[w] - c0)
                    rows = ck * P
                    gt = gpool.tile([P, rows], bf16, tag=f"g{w}")
                    gcol = (woff[w] + c0) * P // 16
                    nc.gpsimd.dma_gather(
                        out_ap=gt[:].rearrange("p (k f) -> p k f", f=P),
                        in_ap=table_ap[w * WIN: w * WIN + wrows[w], :],
                        idxs_ap=gidx_sb[:, gcol: gcol + rows // 16],
                        num_idxs=rows, num_idxs_reg=rows, elem_size=P,
                        queue_num=qn[0] % 4)
                    qn[0] += 1
                    ot = opool.tile([P, rows], bf16, tag=f"o{w}")
                    gs = int(spanoff[w]) + s
                    nc.sync.dma_start(out=ot[:],
                                      in_=oh_d[gs * P:(gs + 1) * P, :rows])
                    span_tiles[w][s] = (gt, ot)

                for b in range(BLOCKS):
                    ps_mt = pspool.tile([P, P], f32, tag="mt")
                    total = int(cbr[b].sum())
                    done = 0
                    for w in range(NWIN):
                        for j in range(int(cbr[b, w])):
                            cg = int(cell_off[w, b]) + j       # global chunk
                            cw = cg - int(woff[w])             # within stream
                            s = cw // SPAN_CHUNKS
                            ensure_span(w, s)
                            gt, ot = span_tiles[w][s]
                            co = cw - s * SPAN_CHUNKS
                            nc.tensor.matmul(
                                ps_mt[:],
                                lhsT=gt[:, co * P:(co + 1) * P],
                                rhs=ot[:, co * P:(co + 1) * P],
                                start=(done == 0), stop=(done == total - 1))
                            done += 1
                    # epilogue: y.T = relu(W.T @ m.T + b); store y
                    mt_sb = mpool.tile([P, P], bf16, tag="mt_sb")
                    nc.scalar.activation(mt_sb[:], ps_mt[:], AF.Copy)
                    ps_yt = pspool.tile([fo, P], f32, tag="yt")
                    nc.tensor.matmul(ps_yt[:], lhsT=w_sb[li][:],
                                     rhs=mt_sb[:], start=True, stop=True)
                    yt_sb = mpool.tile([fo, P], bf16, tag="yt_sb")
                    nc.scalar.activation(
                        yt_sb[:], ps_yt[:],
                        AF.Relu if relu else AF.Identity, bias=b_sb[li][:])
                    ps_y = pspool.tile([P, fo], bf16, tag="y")
                    nc.tensor.transpose(ps_y[:], yt_sb[:],
                                        ident_sb[:fo, :fo])
                    if li < 2:
                        y_sb = mpool.tile([P, fo], bf16, tag="y_sb")
                        nc.scalar.activation(y_sb[:], ps_y[:], AF.Copy)
                        nc.sync.dma_start(out=ag_in[b * P:(b + 1) * P, :],
                                          in_=y_sb[:])
                    else:
                        y_sb = mpool.tile([P, fo], f32, tag="y_sbf")
                        nc.scalar.activation(y_sb[:], ps_y[:], AF.Copy)
                        hi = min((b + 1) * P, SHARD)
                        nc.sync.dma_start(out=out_d[b * P:hi, :],
                                          in_=y_sb[:hi - b * P, :])
                if li < 2:
                    nc.gpsimd.collective_compute(
                        "AllGather", mybir.AluOpType.bypass,
                        replica_groups=[list(range(NCORES))],
                        ins=[ag_in.opt()],
                        outs=[hf[li].opt()],
                    )

            layer(0, feats[:], F_HID, True)
            layer(1, hf[0][:], F_HID, True)
            layer(2, hf[1][:], F_OUT, False)

    nc.compile()
    return nc


def kernel(**inputs):
    global LAST_EXEC_NS, LAST_RESULTS
    from concourse.bass_utils import run_bass_kernel_spmd

    cbr, gidx, onehot = _preprocess(inputs["src"], inputs["dst"])
    nc = _build_program(cbr)

    feats = np.asarray(inputs["features"], dtype=np.float32)
    featpad = np.zeros((TAB_PAD, F_IN), BF16)
    for c in range(NCORES):
        featpad[c * SHARD_PAD: c * SHARD_PAD + SHARD] = \
            feats[c * SHARD: (c + 1) * SHARD].astype(BF16)

    common = {
        "featpad": featpad,
        "W0": np.asarray(inputs["W0"], dtype=np.float32).astype(BF16),
        "W1": np.asarray(inputs["W1"], dtype=np.float32).astype(BF16),
        "W2": np.asarray(inputs["W2"], dtype=np.float32).astype(BF16),
        "b0": np.asarray(inputs["b0"], dtype=np.float32),
        "b1": np.asarray(inputs["b1"], dtype=np.float32),
        "b2": np.asarray(inputs["b2"], dtype=np.float32),
        "identity": np.eye(P, dtype=np.float32).astype(BF16),
    }
    in_maps = []
    for c in range(NCORES):
        m = dict(common)
        m["gidx"] = gidx[c]
        m["onehot"] = onehot[c]
        in_maps.append(m)

    trace = bool(int(os.environ.get("BASS_GNN_TRACE", "0")))
    kwargs = {}
    if trace:
        _register_ntff_hook()
        kwargs = dict(trace=True,
                      tmpdir=os.environ.get("BASS_GNN_TRACE_DIR") or None)
    res = run_bass_kernel_spmd(nc, in_maps, core_ids=list(range(NCORES)),
                               **kwargs)
    LAST_EXEC_NS = res.exec_time_ns
    LAST_RESULTS = res
    out = np.concatenate([res.results[c]["out"] for c in range(NCORES)],
                         axis=0)
    return np.ascontiguousarray(out.astype(np.float32))


def _register_ntff_hook():
    """The container's antenv lacks axon_hooks; register the NTFF profile
    hook ourselves so trace=True works under axon."""
    import sys, types
    if "antenv.axon_hooks" in sys.modules:
        return
    try:
        import antenv
        from trn_agent_boot.trn_boot import _ntff_profile_via_ctypes
        mod = types.ModuleType("antenv.axon_hooks")
        mod._hook = _ntff_profile_via_ctypes('/opt/axon/libaxon_pjrt.so')
        mod.set_axon_ntff_profile_hook = lambda h: setattr(mod, "_hook", h)
        mod.get_axon_ntff_profile_hook = lambda: mod._hook
        sys.modules["antenv.axon_hooks"] = mod
        antenv.axon_hooks = mod
    except Exception as e:
        print("ntff hook registration failed:", e)


# revision 27
# speedup vs baseline: 1.1572x; 1.1572x over previous
"""GraphConv (DGL norm='both', 3 layers) on 8 trn2 NeuronCores.

Sharding: destination nodes (and their edges) are sharded across the 8
cores. Each layer, a core gathers the source rows of its edge shard from
a replicated bf16 node table with dma_gather (SPAN_CHUNKS*128-row spans,
4 SWDGE queues), reduces each 128-edge chunk into a per-dst-block [f, d]
PSUM accumulator via a weighted one-hot matmul (weights carry
ns[src]*nd[dst], so the full symmetric normalization folds into the
segment sum), applies W + bias + relu on the dst shard, and AllGathers
the new bf16 table for the next layer.

The weighted one-hot matrices are data (fixed per graph), so they are
precomputed on the host as one bf16 DRAM stream [128, nchunks*128] and
DMA'd in per span — no per-chunk vector work on device. All tables and
matmul operands are bf16 (fp32 PSUM accumulation); the final output is
fp32.

int16 gather indices only address 32768 rows, so the padded node table
(8 shards x 12544 rows = 100352) is split into 4 windows; each dst
block's edges are bucketed by (window) with a per-(block, window) chunk
count fixed across cores at compile time (schedule rebuilt per call
from the actual graph).
"""

import os
import numpy as np
import ml_dtypes

BF16 = ml_dtypes.bfloat16

N_NODES = 100000
F_IN = 128
F_HID = 128
F_OUT = 64
NCORES = 8
P = 128
SHARD = N_NODES // NCORES          # 12500 dst nodes per core
BLOCKS = (SHARD + P - 1) // P      # 98 dst blocks of 128
SHARD_PAD = BLOCKS * P             # 12544 rows per shard in the table
TAB_PAD = NCORES * SHARD_PAD       # 100352 rows in the gathered table
NWIN = 4                           # index windows (int16 limit 32768 rows)
WIN = TAB_PAD // NWIN              # 25088 rows per window, balanced
SPAN_CHUNKS = 8                    # chunks (of 128 edges) per dma_gather
SPAN_COLS = SPAN_CHUNKS * P        # one-hot columns per span tile
SPAN = SPAN_CHUNKS * P             # rows per dma_gather

LAST_EXEC_NS = None
LAST_RESULTS = None


def _preprocess(src, dst):
    """Build the static schedule and per-core slot arrays.

    Returns (cbr, gidx, onehot):
      cbr    [BLOCKS, NWIN] chunk count per (block, window), shared by cores
      gidx   [NCORES, 128, nslots//16] int16 wrapped gather indices
      onehot [NCORES, 128, nchunks*128] bf16 weighted one-hot blocks:
             onehot[c][p, cg*128 + j] = ns[src]*nd[dst] if slot p of chunk
             cg has local dst j, else 0.
    Slot streams are window-major: for w in windows, for b in blocks,
    cbr[b, w] chunks of 128 edges.
    """
    src = np.asarray(src).astype(np.int64).ravel()
    dst = np.asarray(dst).astype(np.int64).ravel()
    E = src.shape[0]
    deg_out = np.bincount(src, minlength=N_NODES)
    deg_in = np.bincount(dst, minlength=N_NODES)
    ns = np.power(np.maximum(deg_out, 1.0), -0.5).astype(np.float32)
    nd = np.power(np.maximum(deg_in, 1.0), -0.5).astype(np.float32)

    psrc = (src // SHARD) * SHARD_PAD + (src % SHARD)  # padded table row
    win = psrc // WIN
    core = dst // SHARD
    blk = (dst - core * SHARD) // P

    # sort edges by (core, window, block) to match the stream layout
    key = (core * NWIN + win) * BLOCKS + blk
    order = np.argsort(key, kind="stable")
    s_key = key[order]
    s_psrc = psrc[order]
    s_widx = (s_psrc - win[order] * WIN).astype(np.int16)
    s_loc = dst[order] - core[order] * SHARD
    s_dstl = (s_loc % P).astype(np.int64)
    s_w = (ns[src[order]] * nd[dst[order]]).astype(np.float32)

    counts = np.bincount(s_key, minlength=NCORES * NWIN * BLOCKS)
    counts = counts.reshape(NCORES, NWIN, BLOCKS)
    cbr = np.ceil(counts.max(axis=0) / P).astype(np.int64).T  # [BLOCKS, NWIN]
    # a block with no edges at all still needs one (all-zero) chunk so its
    # PSUM accumulator gets written before the epilogue reads it
    empty = cbr.sum(axis=1) == 0
    cbr[empty, 0] = 1
    nchunks = int(cbr.sum())
    nslots = nchunks * P

    # slot offset of each (window, block) cell in the stream
    cell_chunk_off = np.zeros((NWIN, BLOCKS), np.int64)
    off = 0
    for w in range(NWIN):
        for b in range(BLOCKS):
            cell_chunk_off[w, b] = off
            off += cbr[b, w]

    starts = np.concatenate([[0], np.cumsum(counts.reshape(-1))[:-1]])
    pos = np.arange(E, dtype=np.int64) - starts[s_key]
    slot = cell_chunk_off[win[order], blk[order]] * P + pos
    # sanity: every edge fits its cell
    assert (pos < cbr[blk[order], win[order]] * P).all()

    gidx_flat = np.zeros((NCORES, nslots), dtype=np.int16)
    dstl_flat = np.full((NCORES, nslots), 255.0, dtype=np.float32)
    wgt_flat = np.zeros((NCORES, nslots), dtype=np.float32)
    flat = core[order] * nslots + slot
    gidx_flat.reshape(-1)[flat] = s_widx
    dstl_flat.reshape(-1)[flat] = s_dstl.astype(np.float32)
    wgt_flat.reshape(-1)[flat] = s_w

    # gather-index wrap: logical idx i -> [i%16 + 16*rep, i//16]
    gw = gidx_flat.reshape(NCORES, nslots // 16, 16)
    gidx = np.ascontiguousarray(np.tile(gw.transpose(0, 2, 1), (1, 8, 1)))

    # chunk-major [p, chunk] layout for dstl/wgt: slot = chunk*128 + p
    def to_pc(a):
        return np.ascontiguousarray(
            a.reshape(NCORES, nchunks, P).transpose(0, 2, 1))

    return cbr, gidx, to_pc(dstl_flat), to_pc(wgt_flat)


def _build_program(cbr):
    import concourse.bacc as bacc
    import concourse.tile as tile
    from concourse import mybir

    f32 = mybir.dt.float32
    bf16 = mybir.dt.bfloat16
    i16 = mybir.dt.int16
    AF = mybir.ActivationFunctionType
    ALU = mybir.AluOpType

    nchunks = int(cbr.sum())
    nslots = nchunks * P
    # per-window stream extents (in chunks)
    wlen = [int(cbr[:, w].sum()) for w in range(NWIN)]
    woff = np.concatenate([[0], np.cumsum(wlen)]).astype(int)
    wrows = [min(WIN, TAB_PAD - w * WIN) for w in range(NWIN)]
    # chunk offset of cell (w, b) within the global stream
    cell_off = np.zeros((NWIN, BLOCKS), np.int64)
    off = 0
    for w in range(NWIN):
        for b in range(BLOCKS):
            cell_off[w, b] = off
            off += cbr[b, w]

    nc = bacc.Bacc("TRN2", target_bir_lowering=False, debug=False,
                   num_devices=NCORES, num_swdge_queues=4)

    feats = nc.dram_tensor("featpad", [TAB_PAD, F_IN], bf16,
                           kind="ExternalInput")
    w_d = [nc.dram_tensor(f"W{i}", [F_IN, fo], bf16, kind="ExternalInput")
           for i, fo in enumerate([F_HID, F_HID, F_OUT])]
    b_d = [nc.dram_tensor(f"b{i}", [fo], f32, kind="ExternalInput")
           for i, fo in enumerate([F_HID, F_HID, F_OUT])]
    gidx_d = nc.dram_tensor("gidx", [P, nslots // 16], i16,
                            kind="ExternalInput")
    dstl_d = nc.dram_tensor("dstl", [P, nchunks], f32, kind="ExternalInput")
    wgt_d = nc.dram_tensor("wgt", [P, nchunks], f32, kind="ExternalInput")
    iota_d = nc.dram_tensor("iota", [P, P], bf16, kind="ExternalInput")
    ident_d = nc.dram_tensor("identity", [P, P], bf16, kind="ExternalInput")
    out_d = nc.dram_tensor("out", [SHARD, F_OUT], f32, kind="ExternalOutput")

    with tile.TileContext(nc) as tc:
        with (
            tc.tile_pool(name="const", bufs=1) as cpool,
            tc.tile_pool(name="gs", bufs=4) as gpool,
            tc.tile_pool(name="s", bufs=6) as spool,
            tc.tile_pool(name="mid", bufs=3) as mpool,
            tc.tile_pool(name="ps", bufs=2, space="PSUM") as pspool,
            tc.tile_pool(name="dram", bufs=1, space="DRAM") as dpool,
        ):
            iota_sb = cpool.tile([P, P], bf16, tag="iota")
            nc.sync.dma_start(out=iota_sb[:], in_=iota_d[:])
            ident_sb = cpool.tile([P, P], bf16, tag="ident")
            nc.sync.dma_start(out=ident_sb[:], in_=ident_d[:])
            w_sb, b_sb = [], []
            for i, fo in enumerate([F_HID, F_HID, F_OUT]):
                t = cpool.tile([F_IN, fo], bf16, tag=f"w{i}")
                nc.sync.dma_start(out=t[:], in_=w_d[i][:])
                w_sb.append(t)
                t = cpool.tile([fo, 1], f32, tag=f"b{i}")
                nc.sync.dma_start(out=t[:], in_=b_d[i][:, None])
                b_sb.append(t)
            gidx_sb = cpool.tile([P, nslots // 16], i16, tag="gidx")
            nc.sync.dma_start(out=gidx_sb[:], in_=gidx_d[:])
            dstl_sb = cpool.tile([P, nchunks], f32, tag="dstl")
            nc.sync.dma_start(out=dstl_sb[:], in_=dstl_d[:])
            wgt_sb = cpool.tile([P, nchunks], f32, tag="wgt")
            nc.sync.dma_start(out=wgt_sb[:], in_=wgt_d[:])

            ag_in = dpool.tile([SHARD_PAD, F_HID], bf16, tag="ag_in")
            hf0 = dpool.tile([TAB_PAD, F_HID], bf16, tag="hf0")
            hf1 = dpool.tile([TAB_PAD, F_HID], bf16, tag="hf1")
            hf = [hf0, hf1]

            def layer(li, table_ap, fo, relu):
                # spans: per window, chop its chunk stream into
                # <=SPAN_CHUNKS-chunk gathers; span_tiles[w] maps span
                # index -> (gather tile, one-hot tile)
                qn = [0]
                span_tiles = [{} for _ in range(NWIN)]

                def ensure_span(w, s):
                    if s in span_tiles[w]:
                        return
                    c0 = s * SPAN_CHUNKS
                    ck = min(SPAN_CHUNKS, wlen[w] - c0)
                    rows = ck * P
                    gt = gpool.tile([P, rows], bf16, tag=f"g{w}")
                    gcol = (woff[w] + c0) * P // 16
                    nc.gpsimd.dma_gather(
                        out_ap=gt[:].rearrange("p (k f) -> p k f", f=P),
                        in_ap=table_ap[w * WIN: w * WIN + wrows[w], :],
                        idxs_ap=gidx_sb[:, gcol: gcol + rows // 16],
                        num_idxs=rows, num_idxs_reg=rows, elem_size=P,
                        queue_num=qn[0] % 4)
                    qn[0] += 1
                    span_tiles[w][s] = gt

                for b in range(BLOCKS):
                    ps_mt = pspool.tile([P, P], f32, tag="mt")
                    total = int(cbr[b].sum())
                    done = 0
                    for w in range(NWIN):
                        for j in range(int(cbr[b, w])):
                            cg = int(cell_off[w, b]) + j       # global chunk
                            cw = cg - int(woff[w])             # within stream
                            s = cw // SPAN_CHUNKS
                            ensure_span(w, s)
                            gt = span_tiles[w][s]
                            co = cw - s * SPAN_CHUNKS
                            s_t = spool.tile([P, P], bf16, tag="s")
                            nc.vector.tensor_scalar(
                                s_t[:], iota_sb[:],
                                dstl_sb[:, cg:cg + 1],
                                wgt_sb[:, cg:cg + 1],
                                ALU.is_equal, ALU.mult)
                            nc.tensor.matmul(
                                ps_mt[:],
                                lhsT=gt[:, co * P:(co + 1) * P],
                                rhs=s_t[:],
                                start=(done == 0), stop=(done == total - 1))
                            done += 1
                    # epilogue: y.T = relu(W.T @ m.T + b); store y
                    mt_sb = mpool.tile([P, P], bf16, tag="mt_sb")
                    nc.scalar.activation(mt_sb[:], ps_mt[:], AF.Copy)
                    ps_yt = pspool.tile([fo, P], f32, tag="yt")
                    nc.tensor.matmul(ps_yt[:], lhsT=w_sb[li][:],
                                     rhs=mt_sb[:], start=True, stop=True)
                    yt_sb = mpool.tile([fo, P], bf16, tag="yt_sb")
                    nc.scalar.activation(
                        yt_sb[:], ps_yt[:],
                        AF.Relu if relu else AF.Identity, bias=b_sb[li][:])
                    ps_y = pspool.tile([P, fo], bf16, tag="y")
                    nc.tensor.transpose(ps_y[:], yt_sb[:],
                                        ident_sb[:fo, :fo])
                    if li < 2:
                        y_sb = mpool.tile([P, fo], bf16, tag="y_sb")
                        nc.scalar.activation(y_sb[:], ps_y[:], AF.Copy)
                        nc.sync.dma_start(out=ag_in[b * P:(b + 1) * P, :],
                                          in_=y_sb[:])
                    else:
                        y_sb = mpool.tile([P, fo], f32, tag="y_sbf")
                        nc.scalar.activation(y_sb[:], ps_y[:], AF.Copy)
                        hi = min((b + 1) * P, SHARD)
                        nc.sync.dma_start(out=out_d[b * P:hi, :],
                                          in_=y_sb[:hi - b * P, :])
                if li < 2:
                    nc.gpsimd.collective_compute(
                        "AllGather", mybir.AluOpType.bypass,
                        replica_groups=[list(range(NCORES))],
                        ins=[ag_in.opt()],
                        outs=[hf[li].opt()],
                    )

            layer(0, feats[:], F_HID, True)
            layer(1, hf[0][:], F_HID, True)
            layer(2, hf[1][:], F_OUT, False)

    nc.compile()
    return nc


def kernel(**inputs):
    global LAST_EXEC_NS, LAST_RESULTS
    from concourse.bass_utils import run_bass_kernel_spmd

    cbr, gidx, dstl, wgt = _preprocess(inputs["src"], inputs["dst"])
    nc = _build_program(cbr)

    feats = np.asarray(inputs["features"], dtype=np.float32)
    featpad = np.zeros((TAB_PAD, F_IN), BF16)
    for c in range(NCORES):
        featpad[c * SHARD_PAD: c * SHARD_PAD + SHARD] = \
            feats[c * SHARD: (c + 1) * SHARD].astype(BF16)

    common = {
        "featpad": featpad,
        "W0": np.asarray(inputs["W0"], dtype=np.float32).astype(BF16),
        "W1": np.asarray(inputs["W1"], dtype=np.float32).astype(BF16),
        "W2": np.asarray(inputs["W2"], dtype=np.float32).astype(BF16),
        "b0": np.asarray(inputs["b0"], dtype=np.float32),
        "b1": np.asarray(inputs["b1"], dtype=np.float32),
        "b2": np.asarray(inputs["b2"], dtype=np.float32),
        "iota": np.tile(np.arange(P, dtype=np.float32), (P, 1)).astype(BF16),
        "identity": np.eye(P, dtype=np.float32).astype(BF16),
    }
    in_maps = []
    for c in range(NCORES):
        m = dict(common)
        m["gidx"] = gidx[c]
        m["dstl"] = dstl[c]
        m["wgt"] = wgt[c]
        in_maps.append(m)

    trace = bool(int(os.environ.get("BASS_GNN_TRACE", "0")))
    kwargs = {}
    if trace:
        _register_ntff_hook()
        kwargs = dict(trace=True,
                      tmpdir=os.environ.get("BASS_GNN_TRACE_DIR") or None)
    res = run_bass_kernel_spmd(nc, in_maps, core_ids=list(range(NCORES)),
                               **kwargs)
    LAST_EXEC_NS = res.exec_time_ns
    LAST_RESULTS = res
    out = np.concatenate([res.results[c]["out"] for c in range(NCORES)],
                         axis=0)
    return np.ascontiguousarray(out.astype(np.float32))


def _register_ntff_hook():
    """The container's antenv lacks axon_hooks; register the NTFF profile
    hook ourselves so trace=True works under axon."""
    import sys, types
    if "antenv.axon_hooks" in sys.modules:
        return
    try:
        import antenv
        from trn_agent_boot.trn_boot import _ntff_profile_via_ctypes
        mod = types.ModuleType("antenv.axon_hooks")
        mod._hook = _ntff_profile_via_ctypes('/opt/axon/libaxon_pjrt.so')
        mod.set_axon_ntff_profile_hook = lambda h: setattr(mod, "_hook", h)
        mod.get_axon_ntff_profile_hook = lambda: mod._hook
        sys.modules["antenv.axon_hooks"] = mod
        antenv.axon_hooks = mod
    except Exception as e:
        print("ntff hook registration failed:", e)


# revision 29
# speedup vs baseline: 1.2397x; 1.0713x over previous
"""GraphConv (DGL norm='both', 3 layers) on 8 trn2 NeuronCores.

Sharding: destination nodes (and their edges) are sharded across the 8
cores. Each layer, a core gathers the source rows of its edge shard from
a replicated bf16 node table with dma_gather (SPAN_CHUNKS*128-row spans,
4 SWDGE queues), reduces each 128-edge chunk into a per-dst-block [f, d]
PSUM accumulator via a weighted one-hot matmul (weights carry
ns[src]*nd[dst], so the full symmetric normalization folds into the
segment sum), applies W + bias + relu on the dst shard, and AllGathers
the new bf16 table for the next layer.

The weighted one-hot matrices are data (fixed per graph), so they are
precomputed on the host as one bf16 DRAM stream [128, nchunks*128] and
DMA'd in per span — no per-chunk vector work on device. All tables and
matmul operands are bf16 (fp32 PSUM accumulation); the final output is
fp32.

int16 gather indices only address 32768 rows, so the padded node table
(8 shards x 12544 rows = 100352) is split into 4 windows; each dst
block's edges are bucketed by (window) with a per-(block, window) chunk
count fixed across cores at compile time (schedule rebuilt per call
from the actual graph).
"""

import os
import numpy as np
import ml_dtypes

BF16 = ml_dtypes.bfloat16

N_NODES = 100000
F_IN = 128
F_HID = 128
F_OUT = 64
NCORES = 8
P = 128
SHARD = N_NODES // NCORES          # 12500 dst nodes per core
BLOCKS = (SHARD + P - 1) // P      # 98 dst blocks of 128
SHARD_PAD = BLOCKS * P             # 12544 rows per shard in the table
TAB_PAD = NCORES * SHARD_PAD       # 100352 rows in the gathered table
NWIN = 4                           # index windows (int16 limit 32768 rows)
WIN = TAB_PAD // NWIN              # 25088 rows per window, balanced
SPAN_CHUNKS = 4                    # chunks (of 128 edges) per dma_gather
SPAN_COLS = SPAN_CHUNKS * P        # one-hot columns per span tile
SPAN = SPAN_CHUNKS * P             # rows per dma_gather

LAST_EXEC_NS = None
LAST_RESULTS = None


def _preprocess(src, dst):
    """Build the static schedule and per-core slot arrays.

    Returns (cbr, gidx, onehot):
      cbr    [BLOCKS, NWIN] chunk count per (block, window), shared by cores
      gidx   [NCORES, 128, nslots//16] int16 wrapped gather indices
      onehot [NCORES, 128, nchunks*128] bf16 weighted one-hot blocks:
             onehot[c][p, cg*128 + j] = ns[src]*nd[dst] if slot p of chunk
             cg has local dst j, else 0.
    Slot streams are window-major: for w in windows, for b in blocks,
    cbr[b, w] chunks of 128 edges.
    """
    src = np.asarray(src).astype(np.int64).ravel()
    dst = np.asarray(dst).astype(np.int64).ravel()
    E = src.shape[0]
    deg_out = np.bincount(src, minlength=N_NODES)
    deg_in = np.bincount(dst, minlength=N_NODES)
    ns = np.power(np.maximum(deg_out, 1.0), -0.5).astype(np.float32)
    nd = np.power(np.maximum(deg_in, 1.0), -0.5).astype(np.float32)

    psrc = (src // SHARD) * SHARD_PAD + (src % SHARD)  # padded table row
    win = psrc // WIN
    core = dst // SHARD
    blk = (dst - core * SHARD) // P

    # sort edges by (core, window, block) to match the stream layout
    key = (core * NWIN + win) * BLOCKS + blk
    order = np.argsort(key, kind="stable")
    s_key = key[order]
    s_psrc = psrc[order]
    s_widx = (s_psrc - win[order] * WIN).astype(np.int16)
    s_loc = dst[order] - core[order] * SHARD
    s_dstl = (s_loc % P).astype(np.int64)
    s_w = (ns[src[order]] * nd[dst[order]]).astype(np.float32)

    counts = np.bincount(s_key, minlength=NCORES * NWIN * BLOCKS)
    counts = counts.reshape(NCORES, NWIN, BLOCKS)
    cbr = np.ceil(counts.max(axis=0) / P).astype(np.int64).T  # [BLOCKS, NWIN]
    # a block with no edges at all still needs one (all-zero) chunk so its
    # PSUM accumulator gets written before the epilogue reads it
    empty = cbr.sum(axis=1) == 0
    cbr[empty, 0] = 1
    nchunks = int(cbr.sum())
    nslots = nchunks * P

    # slot offset of each (window, block) cell in the stream
    cell_chunk_off = np.zeros((NWIN, BLOCKS), np.int64)
    off = 0
    for w in range(NWIN):
        for b in range(BLOCKS):
            cell_chunk_off[w, b] = off
            off += cbr[b, w]

    starts = np.concatenate([[0], np.cumsum(counts.reshape(-1))[:-1]])
    pos = np.arange(E, dtype=np.int64) - starts[s_key]
    slot = cell_chunk_off[win[order], blk[order]] * P + pos
    # sanity: every edge fits its cell
    assert (pos < cbr[blk[order], win[order]] * P).all()

    gidx_flat = np.zeros((NCORES, nslots), dtype=np.int16)
    dstl_flat = np.full((NCORES, nslots), 255.0, dtype=np.float32)
    wgt_flat = np.zeros((NCORES, nslots), dtype=np.float32)
    flat = core[order] * nslots + slot
    gidx_flat.reshape(-1)[flat] = s_widx
    dstl_flat.reshape(-1)[flat] = s_dstl.astype(np.float32)
    wgt_flat.reshape(-1)[flat] = s_w

    # gather-index wrap: logical idx i -> [i%16 + 16*rep, i//16]
    gw = gidx_flat.reshape(NCORES, nslots // 16, 16)
    gidx = np.ascontiguousarray(np.tile(gw.transpose(0, 2, 1), (1, 8, 1)))

    # chunk-major [p, chunk] layout for dstl/wgt: slot = chunk*128 + p
    def to_pc(a):
        return np.ascontiguousarray(
            a.reshape(NCORES, nchunks, P).transpose(0, 2, 1))

    return cbr, gidx, to_pc(dstl_flat), to_pc(wgt_flat)


def _build_program(cbr):
    import concourse.bacc as bacc
    import concourse.tile as tile
    from concourse import mybir

    f32 = mybir.dt.float32
    bf16 = mybir.dt.bfloat16
    i16 = mybir.dt.int16
    AF = mybir.ActivationFunctionType
    ALU = mybir.AluOpType

    nchunks = int(cbr.sum())
    nslots = nchunks * P
    # per-window stream extents (in chunks)
    wlen = [int(cbr[:, w].sum()) for w in range(NWIN)]
    woff = np.concatenate([[0], np.cumsum(wlen)]).astype(int)
    wrows = [min(WIN, TAB_PAD - w * WIN) for w in range(NWIN)]
    # chunk offset of cell (w, b) within the global stream
    cell_off = np.zeros((NWIN, BLOCKS), np.int64)
    off = 0
    for w in range(NWIN):
        for b in range(BLOCKS):
            cell_off[w, b] = off
            off += cbr[b, w]

    nc = bacc.Bacc("TRN2", target_bir_lowering=False, debug=False,
                   num_devices=NCORES, num_swdge_queues=4)

    feats = nc.dram_tensor("featpad", [TAB_PAD, F_IN], bf16,
                           kind="ExternalInput")
    w_d = [nc.dram_tensor(f"W{i}", [F_IN, fo], bf16, kind="ExternalInput")
           for i, fo in enumerate([F_HID, F_HID, F_OUT])]
    b_d = [nc.dram_tensor(f"b{i}", [fo], f32, kind="ExternalInput")
           for i, fo in enumerate([F_HID, F_HID, F_OUT])]
    gidx_d = nc.dram_tensor("gidx", [P, nslots // 16], i16,
                            kind="ExternalInput")
    dstl_d = nc.dram_tensor("dstl", [P, nchunks], f32, kind="ExternalInput")
    wgt_d = nc.dram_tensor("wgt", [P, nchunks], f32, kind="ExternalInput")
    iota_d = nc.dram_tensor("iota", [P, P], bf16, kind="ExternalInput")
    ident_d = nc.dram_tensor("identity", [P, P], bf16, kind="ExternalInput")
    out_d = nc.dram_tensor("out", [SHARD, F_OUT], f32, kind="ExternalOutput")

    with tile.TileContext(nc) as tc:
        with (
            tc.tile_pool(name="const", bufs=1) as cpool,
            tc.tile_pool(name="gs", bufs=4) as gpool,
            tc.tile_pool(name="s", bufs=6) as spool,
            tc.tile_pool(name="mid", bufs=3) as mpool,
            tc.tile_pool(name="ps", bufs=2, space="PSUM") as pspool,
            tc.tile_pool(name="dram", bufs=1, space="DRAM") as dpool,
        ):
            iota_sb = cpool.tile([P, P], bf16, tag="iota")
            nc.sync.dma_start(out=iota_sb[:], in_=iota_d[:])
            ident_sb = cpool.tile([P, P], bf16, tag="ident")
            nc.sync.dma_start(out=ident_sb[:], in_=ident_d[:])
            w_sb, b_sb = [], []
            for i, fo in enumerate([F_HID, F_HID, F_OUT]):
                t = cpool.tile([F_IN, fo], bf16, tag=f"w{i}")
                nc.sync.dma_start(out=t[:], in_=w_d[i][:])
                w_sb.append(t)
                t = cpool.tile([fo, 1], f32, tag=f"b{i}")
                nc.sync.dma_start(out=t[:], in_=b_d[i][:, None])
                b_sb.append(t)
            gidx_sb = cpool.tile([P, nslots // 16], i16, tag="gidx")
            nc.sync.dma_start(out=gidx_sb[:], in_=gidx_d[:])
            dstl_sb = cpool.tile([P, nchunks], f32, tag="dstl")
            nc.sync.dma_start(out=dstl_sb[:], in_=dstl_d[:])
            wgt_sb = cpool.tile([P, nchunks], f32, tag="wgt")
            nc.sync.dma_start(out=wgt_sb[:], in_=wgt_d[:])

            ag_in = dpool.tile([SHARD_PAD, F_HID], bf16, tag="ag_in")
            hf0 = dpool.tile([TAB_PAD, F_HID], bf16, tag="hf0")
            hf1 = dpool.tile([TAB_PAD, F_HID], bf16, tag="hf1")
            hf = [hf0, hf1]

            def layer(li, table_ap, fo, relu):
                # spans: per window, chop its chunk stream into
                # <=SPAN_CHUNKS-chunk gathers; span_tiles[w] maps span
                # index -> (gather tile, one-hot tile)
                qn = [0]
                span_tiles = [{} for _ in range(NWIN)]

                def ensure_span(w, s):
                    if s in span_tiles[w]:
                        return
                    c0 = s * SPAN_CHUNKS
                    ck = min(SPAN_CHUNKS, wlen[w] - c0)
                    rows = ck * P
                    gt = gpool.tile([P, rows], bf16, tag=f"g{w}")
                    gcol = (woff[w] + c0) * P // 16
                    nc.gpsimd.dma_gather(
                        out_ap=gt[:].rearrange("p (k f) -> p k f", f=P),
                        in_ap=table_ap[w * WIN: w * WIN + wrows[w], :],
                        idxs_ap=gidx_sb[:, gcol: gcol + rows // 16],
                        num_idxs=rows, num_idxs_reg=rows, elem_size=P,
                        queue_num=qn[0] % 4)
                    qn[0] += 1
                    span_tiles[w][s] = gt

                for b in range(BLOCKS):
                    ps_mt = pspool.tile([P, P], f32, tag="mt")
                    total = int(cbr[b].sum())
                    done = 0
                    for w in range(NWIN):
                        for j in range(int(cbr[b, w])):
                            cg = int(cell_off[w, b]) + j       # global chunk
                            cw = cg - int(woff[w])             # within stream
                            s = cw // SPAN_CHUNKS
                            ensure_span(w, s)
                            gt = span_tiles[w][s]
                            co = cw - s * SPAN_CHUNKS
                            s_t = spool.tile([P, P], bf16, tag="s")
                            nc.vector.tensor_scalar(
                                s_t[:], iota_sb[:],
                                dstl_sb[:, cg:cg + 1],
                                wgt_sb[:, cg:cg + 1],
                                ALU.is_equal, ALU.mult)
                            nc.tensor.matmul(
                                ps_mt[:],
                                lhsT=gt[:, co * P:(co + 1) * P],
                                rhs=s_t[:],
                                start=(done == 0), stop=(done == total - 1))
                            done += 1
                    # epilogue: y.T = relu(W.T @ m.T + b); store y
                    mt_sb = mpool.tile([P, P], bf16, tag="mt_sb")
                    nc.scalar.activation(mt_sb[:], ps_mt[:], AF.Copy)
                    ps_yt = pspool.tile([fo, P], f32, tag="yt")
                    nc.tensor.matmul(ps_yt[:], lhsT=w_sb[li][:],
                                     rhs=mt_sb[:], start=True, stop=True)
                    yt_sb = mpool.tile([fo, P], bf16, tag="yt_sb")
                    nc.scalar.activation(
                        yt_sb[:], ps_yt[:],
                        AF.Relu if relu else AF.Identity, bias=b_sb[li][:])
                    ps_y = pspool.tile([P, fo], bf16, tag="y")
                    nc.tensor.transpose(ps_y[:], yt_sb[:],
                                        ident_sb[:fo, :fo])
                    if li < 2:
                        y_sb = mpool.tile([P, fo], bf16, tag="y_sb")
                        nc.scalar.activation(y_sb[:], ps_y[:], AF.Copy)
                        nc.sync.dma_start(out=ag_in[b * P:(b + 1) * P, :],
                                          in_=y_sb[:])
                    else:
                        y_sb = mpool.tile([P, fo], f32, tag="y_sbf")
                        nc.scalar.activation(y_sb[:], ps_y[:], AF.Copy)
                        hi = min((b + 1) * P, SHARD)
                        nc.sync.dma_start(out=out_d[b * P:hi, :],
                                          in_=y_sb[:hi - b * P, :])
                if li < 2:
                    nc.gpsimd.collective_compute(
                        "AllGather", mybir.AluOpType.bypass,
                        replica_groups=[list(range(NCORES))],
                        ins=[ag_in.opt()],
                        outs=[hf[li].opt()],
                    )

            layer(0, feats[:], F_HID, True)
            layer(1, hf[0][:], F_HID, True)
            layer(2, hf[1][:], F_OUT, False)

    nc.compile()
    return nc


def kernel(**inputs):
    global LAST_EXEC_NS, LAST_RESULTS
    from concourse.bass_utils import run_bass_kernel_spmd

    cbr, gidx, dstl, wgt = _preprocess(inputs["src"], inputs["dst"])
    nc = _build_program(cbr)

    feats = np.asarray(inputs["features"], dtype=np.float32)
    featpad = np.zeros((TAB_PAD, F_IN), BF16)
    for c in range(NCORES):
        featpad[c * SHARD_PAD: c * SHARD_PAD + SHARD] = \
            feats[c * SHARD: (c + 1) * SHARD].astype(BF16)

    common = {
        "featpad": featpad,
        "W0": np.asarray(inputs["W0"], dtype=np.float32).astype(BF16),
        "W1": np.asarray(inputs["W1"], dtype=np.float32).astype(BF16),
        "W2": np.asarray(inputs["W2"], dtype=np.float32).astype(BF16),
        "b0": np.asarray(inputs["b0"], dtype=np.float32),
        "b1": np.asarray(inputs["b1"], dtype=np.float32),
        "b2": np.asarray(inputs["b2"], dtype=np.float32),
        "iota": np.tile(np.arange(P, dtype=np.float32), (P, 1)).astype(BF16),
        "identity": np.eye(P, dtype=np.float32).astype(BF16),
    }
    in_maps = []
    for c in range(NCORES):
        m = dict(common)
        m["gidx"] = gidx[c]
        m["dstl"] = dstl[c]
        m["wgt"] = wgt[c]
        in_maps.append(m)

    trace = bool(int(os.environ.get("BASS_GNN_TRACE", "0")))
    kwargs = {}
    if trace:
        _register_ntff_hook()
        kwargs = dict(trace=True,
                      tmpdir=os.environ.get("BASS_GNN_TRACE_DIR") or None)
    res = run_bass_kernel_spmd(nc, in_maps, core_ids=list(range(NCORES)),
                               **kwargs)
    LAST_EXEC_NS = res.exec_time_ns
    LAST_RESULTS = res
    out = np.concatenate([res.results[c]["out"] for c in range(NCORES)],
                         axis=0)
    return np.ascontiguousarray(out.astype(np.float32))


def _register_ntff_hook():
    """The container's antenv lacks axon_hooks; register the NTFF profile
    hook ourselves so trace=True works under axon."""
    import sys, types
    if "antenv.axon_hooks" in sys.modules:
        return
    try:
        import antenv
        from trn_agent_boot.trn_boot import _ntff_profile_via_ctypes
        mod = types.ModuleType("antenv.axon_hooks")
        mod._hook = _ntff_profile_via_ctypes('/opt/axon/libaxon_pjrt.so')
        mod.set_axon_ntff_profile_hook = lambda h: setattr(mod, "_hook", h)
        mod.get_axon_ntff_profile_hook = lambda: mod._hook
        sys.modules["antenv.axon_hooks"] = mod
        antenv.axon_hooks = mod
    except Exception as e:
        print("ntff hook registration failed:", e)
